# revision 65
# baseline (speedup 1.0000x reference)
"""Multi-head attention (B=2, N=2048, DIM=1024, H=16) on 8 Trainium2 NeuronCores.

Sharding: tensor-parallel by head within two quads (cores 0-3 -> batch 0,
cores 4-7 -> batch 1; quad rank r owns heads 4r..4r+3). Each core computes
Q/K/V projections for its 4 heads and masked-softmax attention; per-quad
(4-core) AllToAlls re-shard the attention output from head-split to
sequence-split; each core runs the output projection (+bias) for a disjoint
512-token slice. The host only shards inputs (transpose + bf16 cast) and
concatenates the 8 output shards.

v3 structure (vs the 461us v2):
  - quad-local AllToAlls (replica groups [[0-3],[4-7]]): half the collective
    payload, and the output projection contracts only the 8 real source
    blocks (v2 contracted 16, half against zero-padded weights).
  - the a2a payload is the UNNORMALIZED attention sum plus the softmax
    denominator row (the extra ones-column of the attn@V matmul);
    normalization happens on the receiver with reciprocal_approx_fast +
    a tiny selector-matmul partition-broadcast. This removes the whole
    normalization chain (reciprocals were 56us of DVE) from the attention
    passes and lets each a2a launch one pass earlier.
  - merged exp: the two heads of a pass share one [128,1024] PSUM score
    pair (2 banks), so ExpE runs half as many, double-size activations.
  - receiver-side prep for the first a2a is interleaved into the later
    attention passes; the tail is only the second a2a + projection matmuls.

Numerics: matmuls in bf16 with fp32 PSUM accumulation; softmax computed as
exp(SCALE*S)*mask / sum(exp(SCALE*S)*mask) without max-subtraction (scores
are ~N(0,1) after SCALE; exp never overflows). Denominators come from an
extra ones-column appended to V in the attn@V matmul (column 64 of each
head's [128,65] V tile); they travel through the a2a in bf16 (0.4% rel
error, well under the 2e-2 gate).
"""

import numpy as np
import ml_dtypes

import concourse.bass as bass
import concourse.mybir as mybir
import concourse.tile as tile

F32 = mybir.dt.float32
BF16 = mybir.dt.bfloat16
BF16_NP = ml_dtypes.bfloat16

B, DIM, H = 2, 1024, 16
N_FULL = 2048
HD = DIM // H          # 64
SCALE = HD ** -0.5     # 0.125
NCORES = 8
H_LOC = H // 4         # 4 heads per core
COLS = H_LOC * HD      # 256 local channels
KT_D = DIM // 128      # 8 contraction tiles over DIM
GROUPS = [list(range(NCORES))]   # NRT mesh needs >4 cores per group
NQ = 4                 # quad size
USE_MASK_BCAST = True  # single [128,2,HS] mask multiply via stride-0 view


# ---------------------------------------------------------------------------
# Workaround: this walrus build rejects >2 sync waits on one instruction
# ("Too many sync wait commands" in setupSyncWait). The TileContext final
# drain aggregates one wait per logical processor; split it into a chain of
# single-wait drains.
# ---------------------------------------------------------------------------
def _patch_tile_drain():
    from bass_rust import ScopedClock

    if getattr(tile.TileContext, "_drain_patched", False):
        return

    def _drain_and_barrier(self, tick_clock, wait_clock):
        nc = self.nc
        drain_inst = nc.sync.drain()
        wait_clock.add_sem_waits(
            drain_inst.ins, ScopedClock({None: tick_clock.global_clock})
        )
        si = drain_inst.ins.sync_info
        if si is not None and len(si.on_wait) > 1:
            waits = list(si.on_wait)
            drain_inst.ins.sync_info = mybir.SyncInfo(
                on_wait=waits[:1], on_update=list(si.on_update)
            )
            for w in waits[1:]:
                d = nc.sync.drain()
                dsi = d.ins.sync_info
                upd = list(dsi.on_update) if dsi is not None else []
                d.ins.sync_info = mybir.SyncInfo(on_wait=[w], on_update=upd)

        nc.all_engine_barrier()
        assert self.sems is not None
        popped = nc._tile_sem_poison_stack.pop()
        assert popped is self._sem_poison
        nc.clear_and_free_semaphores(list(self.sems.allocated().values()))
        nc.all_engine_barrier()

    tile.TileContext._drain_and_barrier = _drain_and_barrier
    tile.TileContext._drain_patched = True


def _split_sync_waits(nc, maxw=1):
    """Walrus in this build rejects instructions carrying more than a couple
    of semaphore waits. Move excess waits onto injected same-engine NoOps
    immediately before the instruction (identical semantics: the engine
    blocks at the nop instead of at the instruction itself)."""
    n_split = 0
    for f in nc.m.functions:
        for bb in f.blocks:
            new_insts = []
            for ins in bb.instructions:
                si = ins.sync_info
                if si is not None and len(si.on_wait) > maxw:
                    waits = list(si.on_wait)
                    for i, w in enumerate(waits[maxw:]):
                        nop = mybir.InstNoOp(
                            name=f"{ins.name}-w{i}", ins=[], outs=[]
                        )
                        nop.engine = ins.engine
                        nop.sync_info = mybir.SyncInfo(
                            on_wait=[w], on_update=[]
                        )
                        new_insts.append(nop)
                    ins.sync_info = mybir.SyncInfo(
                        on_wait=waits[:maxw], on_update=list(si.on_update)
                    )
                    n_split += 1
                new_insts.append(ins)
            bb.instructions = new_insts
    return n_split


def build_nc(N=N_FULL, split_waits=True):
    """Build the per-core Bass program (same SPMD program for all 8 cores).

    N is parameterizable (multiple of 512) so a scaled-down variant can be
    validated in the simulator; the graded configuration is N=2048.
    """
    _patch_tile_drain()
    assert N % 512 == 0
    NSLICE = N // 4            # output rows per core
    MT = N // 128              # m-tiles over keys
    HS = 512                   # attention n-chunk size
    NH = N // HS               # number of n-chunks per head pair
    NT = NSLICE // 128         # output row tiles
    NCH = N // 512             # 512-col chunks of N

    nc = bass.Bass(trn_type="TRN2", num_devices=NCORES)

    # x_q host-packed as [p, nch, half, kt4, n] so each Q-chunk DMA is
    # contiguous per partition (column-sliced reads of x^T would move in
    # 1KB granules and run ~4x slower)
    xqr_e = nc.declare_dram_parameter("xqr", [128, KT_D * N], BF16,
                                      isOutput=False)
    xkT_e = nc.declare_dram_parameter("xkT", [DIM, N], BF16, isOutput=False)
    # weights host-rearranged to [p, kt, c] so their DMA is contiguous
    wq_e = nc.declare_dram_parameter("wq", [128, KT_D * COLS], BF16,
                                     isOutput=False)
    wk_e = nc.declare_dram_parameter("wk", [128, KT_D * COLS], BF16,
                                     isOutput=False)
    wv_e = nc.declare_dram_parameter("wv", [128, KT_D * COLS], BF16,
                                     isOutput=False)
    # x_v host-rearranged to [p, kt, n] (contiguous DMA)
    xvr_e = nc.declare_dram_parameter("xvr", [128, KT_D * N], BF16,
                                      isOutput=False)
    # Wp rows permuted host-side into [(s,j) block, 128, DIM]: block (s,j)
    # holds the rows for source quad-rank j's head pair s. Source cores j
    # and j+4 share the block (same heads, different batch) -- the per-core
    # `sel` selector zeroes the cross-quad (wrong-batch) contributions.
    wpp_e = nc.declare_dram_parameter("wp_perm", [DIM, DIM], BF16,
                                      isOutput=False)
    # per-partition quad/batch filter for the denominator reciprocals: row
    # 2g+h is 1.0 iff source core g is in this core's quad. Per-core data;
    # the program stays SPMD.
    selc_e = nc.declare_dram_parameter("selc", [16, 1], F32, isOutput=False)
    maskT_e = nc.declare_dram_parameter("maskT", [N, N], BF16, isOutput=False)
    bpr_e = nc.declare_dram_parameter("bp_rep", [128, DIM], F32, isOutput=False)
    # bf16 output (0.2% quantization, well under the 2e-2 gate) halves the
    # output-DMA bytes on the tail critical path; the host upcasts
    out_e = nc.declare_dram_parameter("out", [NSLICE, DIM], BF16,
                                      isOutput=True)

    # per-head-pair AllToAll buffers: 8 peers x (2 heads x 65 rows); the
    # chunk for peers d and d+4 carries the same payload (only the same-quad
    # copy is used downstream).
    a2a_in = [nc.dram_tensor(f"a2a_in{s}", [NCORES * 130, NSLICE], BF16)
              for s in range(2)]
    a2a_out = [nc.dram_tensor(f"a2a_out{s}", [NCORES * 130, NSLICE], BF16)
               for s in range(2)]
    # DRAM staging for the zeroed denominator reciprocals: a stride-0 DMA
    # from here broadcasts each row over 64 SBUF partitions
    drec = [nc.dram_tensor(f"drec{s}", [16, NSLICE], BF16) for s in range(2)]

    with tile.TileContext(nc) as tc:
        with (
            tc.tile_pool(name="cpool", bufs=1) as cpool,
            tc.tile_pool(name="xstream", bufs=3) as xpool,
            tc.tile_pool(name="pupool", bufs=3) as pupool,
            tc.tile_pool(name="pumpool", bufs=4) as pumpool,
            tc.tile_pool(name="evpool", bufs=2) as evpool,
            tc.tile_pool(name="p3pool", bufs=2) as p3pool,
            tc.tile_pool(name="opool", bufs=2) as opool,
            tc.tile_pool(name="ps", bufs=1, space="PSUM") as ps,
        ):
            # PSUM: 16KB/partition as five slots -- PA/PB/PC are 2-bank
            # [128,1024] slots, PD1/PD2 single banks. Phase 1 uses all five
            # as projection accumulators; phase 2 uses PA/PB as the 2-deep
            # score-pair rotation (both heads side by side, so one Exp covers
            # them), PC as the attn@V accumulator pair, PD1/PD2 for the
            # interleaved receiver-side normalization broadcasts; phase 3
            # uses PA/PB/PC as output-projection accumulators and PD1/PD2
            # for the last row tile.
            # ---- long-lived SBUF tensors -------------------------------
            qt_sb = [cpool.tile([128, N], BF16, tag=f"qt{i}", name=f"qt{i}")
                     for i in range(2)]
            kt_sb = [cpool.tile([128, N], BF16, tag=f"kt{i}", name=f"kt{i}")
                     for i in range(2)]
            # V per m-tile: [m, head, 65]; cols 0..63 = V_head, col 64 = ones
            vt_sb = [cpool.tile([128, H_LOC, 65], BF16, tag=f"vt{t}",
                                name=f"vt{t}")
                     for t in range(MT)]
            mask_sb = cpool.tile([128, MT, N], BF16, tag="mask", name="mask")
            bpr_sb = cpool.tile([128, DIM], F32, tag="bpr", name="bpr")
            wq_sb = cpool.tile([128, KT_D, COLS], BF16, tag="wq", name="wq")
            wk_sb = cpool.tile([128, KT_D, COLS], BF16, tag="wk", name="wk")
            wv_sb = cpool.tile([128, KT_D, COLS], BF16, tag="wv", name="wv")
            xv_sb = cpool.tile([128, KT_D, N], BF16, tag="xv", name="xv")
            selc_sb = cpool.tile([16, 1], F32, tag="selc", name="selc")
            # phase-3 per-(s,src) normalized x, batched denominator recips,
            # and the 8 distinct Wp blocks
            xn_sb = [cpool.tile([128, NCORES, NSLICE], BF16, tag=f"xn{s}",
                                name=f"xn{s}") for s in range(2)]
            recb_sb = [cpool.tile([16, NSLICE], BF16, tag=f"recb{s}",
                                  name=f"recb{s}") for s in range(2)]
            # the 8 Wp blocks live in xv_sb's space (xv is dead after the
            # V projection; subtile WAR deps order the overwrite)
            if KT_D * N >= 8 * DIM:
                xv_flat = xv_sb[:].rearrange("p k n -> p (k n)")
                wp_sb = [[xv_flat[:, (s * NQ + j) * DIM:
                                  (s * NQ + j + 1) * DIM]
                          for j in range(NQ)] for s in range(2)]
            else:   # scaled-down sim configs have SBUF to spare
                wp_sb = [[cpool.tile([128, DIM], BF16, tag=f"wp{s}{j}",
                                     name=f"wp{s}{j}")
                          for j in range(NQ)] for s in range(2)]

            # weights + constants (wq prefetches during the K round; wv/bpr
            # are emitted later so they don't delay the x streams)
            nc.sync.dma_start(wk_sb[:], wk_e[:].rearrange("p (kt c) -> p kt c", kt=KT_D))
            nc.sync.dma_start(wq_sb[:], wq_e[:].rearrange("p (kt c) -> p kt c", kt=KT_D))
            nc.sync.dma_start(selc_sb[:], selc_e[:])
            for t in range(MT):
                nc.gpsimd.memset(vt_sb[t][:, :, 64:65], 1.0)

            # ---- phase 1: projections ----------------------------------
            # PSUM accumulator slots: (tag, shape, col-offset) covering the
            # 8 [128,512] K/Q accumulators (cb x nch).
            def qk_psum_slots():
                tiles = {}
                def slot(i):
                    tag, off = [("PA", 0), ("PA", 512), ("PB", 0), ("PB", 512),
                                ("PC", 0), ("PC", 512), ("PD", 0), ("PD", 512)][i]
                    if tag not in tiles:
                        tiles[tag] = ps.tile([128, 1024], F32, tag=tag,
                                             name="p1qk")
                    return tiles[tag][:, off:off + 512]
                return slot

            # K^T: [COLS, N] as two 128-row blocks; kt-outer with one live
            # [128, 512] psum accumulator per (block, n-chunk).
            slot = qk_psum_slots()
            psums = [slot(cb * NCH + nch)
                     for cb in range(2) for nch in range(NCH)]
            for kt in range(KT_D):
                xt_t = xpool.tile([128, N], BF16, tag="xs", name="xs")
                nc.sync.dma_start(xt_t[:], xkT_e[128 * kt:128 * (kt + 1), :])
                for cb in range(2):
                    for nch in range(NCH):
                        nc.tensor.matmul(
                            psums[cb * NCH + nch],
                            wk_sb[:, kt, 128 * cb:128 * (cb + 1)],
                            xt_t[:, 512 * nch:512 * (nch + 1)],
                            start=(kt == 0), stop=(kt == KT_D - 1),
                        )
            for cb in range(2):
                for nch in range(NCH):
                    nc.scalar.copy(
                        kt_sb[cb][:, 512 * nch:512 * (nch + 1)],
                        psums[cb * NCH + nch],
                    )

            # Q^T: n-chunk-outer (x re-read as column slices, same total
            # bytes) so chunk 0 is ready right after K/V and the attention
            # passes can start; chunks k>=1 are emitted inside pass (0,k-1)
            # where PD1/PD2 and tensor slack are free.
            def q_chunk(nch):
                qps = ps.tile([128, 1024], F32, tag="PD", name="qps")
                for half in range(2):
                    xq_t = xpool.tile([128, KT_D // 2, 512], BF16, tag="xs",
                                      name="xq")
                    blk = 2 * nch + half
                    nc.sync.dma_start(
                        xq_t[:],
                        xqr_e[:, 2048 * blk:2048 * (blk + 1)].rearrange(
                            "p (kt n) -> p kt n", kt=KT_D // 2
                        ),
                    )
                    for cb in range(2):
                        for k4 in range(KT_D // 2):
                            kt = half * (KT_D // 2) + k4
                            nc.tensor.matmul(
                                qps[:, 512 * cb:512 * (cb + 1)],
                                wq_sb[:, kt, 128 * cb:128 * (cb + 1)],
                                xq_t[:, k4, :],
                                start=(kt == 0), stop=(kt == KT_D - 1),
                            )
                for cb in range(2):
                    with nc.allow_low_precision(reason="qt bf16"):
                        nc.vector.tensor_copy(
                            qt_sb[cb][:, 512 * nch:512 * (nch + 1)],
                            qps[:, 512 * cb:512 * (cb + 1)],
                        )

            q_chunk(0)

            # V in natural layout: out[m-tile, 4*HD] = xvT_kt^T @ wv_kt
            nc.sync.dma_start(
                wv_sb[:], wv_e[:].rearrange("p (kt c) -> p kt c", kt=KT_D)
            )
            # per-kt slices so the V matmuls start as soon as the first
            # contraction slice lands instead of after the whole 4MB
            for kt in range(KT_D):
                nc.sync.dma_start(
                    xv_sb[:, kt, :], xvr_e[:, N * kt:N * (kt + 1)]
                )
            for t in range(MT):
                nc.sync.dma_start(
                    mask_sb[:, t, :], maskT_e[128 * t:128 * (t + 1), :]
                )
            nc.sync.dma_start(bpr_sb[:], bpr_e[:])
            for t in range(MT):
                vps = ps.tile([128, COLS], F32, tag=("PA", "PB")[t % 2],
                              name="p1v")
                for kt in range(KT_D):
                    nc.tensor.matmul(
                        vps[:],
                        xv_sb[:, kt, 128 * t:128 * (t + 1)],
                        wv_sb[:, kt, :],
                        start=(kt == 0), stop=(kt == KT_D - 1),
                    )
                nc.scalar.copy(
                    vt_sb[t][:, :, 0:HD],
                    vps[:].rearrange("p (h d) -> p h d", h=H_LOC),
                )

            # phase-3 Wp blocks: DMA them up front (deadline is the tail)
            wpp_v = wpp_e[:].rearrange("(b p) c -> b p c", p=128)
            for s in range(2):
                for j in range(NQ):
                    nc.sync.dma_start(wp_sb[s][j][:], wpp_v[s * NQ + j])

            # ---- receiver-side prep for the a2a payload ----------------
            # prep_dn_a: one strided DMA gathers all 16 denominator rows
            # (every 65th row of a2a_out) + one batched [16,NSLICE]
            # reciprocal (replaces the baseline's 32 single-partition
            # reciprocals, 55us of DVE).
            # prep_dn_b: downcast with the per-partition quad filter (cross-
            # quad rows -> 0) and stage to DRAM for the broadcast DMAs.
            # prep_block: one DMA for the block's 128 x rows, one stride-0
            # broadcast DMA for its two recip rows, one 2x-mode multiply.
            # prep_sum folds the (wrong-batch, zeroed) twin into the
            # quad-local tile so the projection contracts 8 blocks.
            # prep DMAs issue from the (otherwise idle) GpSimd queue: if the
            # collective runs long, only GpSimd stalls on it -- the Sync
            # queue keeps issuing the passes' staging DMAs. The DVE ops are
            # pinned to the end of the schedule with tile_wait_until: the
            # Tile scheduler's cost model assumes collectives are fast and
            # would otherwise hoist them into the middle of the attention
            # passes, where their data dependency on the in-flight a2a
            # stalls the in-order DVE queue (and with it exp/mask/attnV).
            def W(ms):
                return tc.tile_wait_until(ms)

            def preps(s, w):
                # batched receiver-side prep: 1 strided DMA for the 16
                # denominator rows, one [16,NSLICE] reciprocal (vs the
                # baseline's 32 single-partition ones), zeroed bf16 downcast
                # (per-partition quad filter), DRAM roundtrip, then two
                # 4-block rounds of {x halves + stride-0 recip broadcast}
                # DMAs, each closed by one [128,4,NSLICE] multiply.
                srcv = a2a_out[s][:].rearrange("(q r) n -> q r n", r=65)
                srcp = srcv.rearrange("q r n -> r q n")
                dn = p3pool.tile([16, NSLICE], BF16, tag="dn", name="dn",
                                 bufs=1)
                rec = p3pool.tile([16, NSLICE], F32, tag="rec", name="rec",
                                  bufs=1)
                with W(w):
                    nc.scalar.dma_start(dn[:], srcv[:, 64, :])
                    nc.vector.reciprocal(rec[:], dn[:])
                    with nc.allow_low_precision(reason="softmax recip bf16"):
                        nc.vector.tensor_scalar_mul(recb_sb[s][:], rec[:],
                                                    selc_sb[:])
                    aa0 = p3pool.tile([128, NQ, NSLICE], BF16, tag="aa",
                                      name="aa", bufs=1)
                    for h in range(2):   # x stream starts during the recip
                        nc.scalar.dma_start(
                            aa0[64 * h:64 * (h + 1), :, :],
                            srcp[0:64, slice(h, 8, 2), :],
                        )
                    nc.scalar.dma_start(drec[s][:], recb_sb[s][:])
                    for q in range(2):
                        aa = aa0 if q == 0 else p3pool.tile(
                            [128, NQ, NSLICE], BF16, tag="aa", name="aa",
                            bufs=1)
                        rb = p3pool.tile([128, NQ, NSLICE], BF16, tag="rb",
                                         name="rb", bufs=1)
                        for h in range(2):
                            qs = slice(8 * q + h, 8 * q + 8, 2)
                            if q == 1:
                                nc.scalar.dma_start(
                                    aa[64 * h:64 * (h + 1), :, :],
                                    srcp[0:64, qs, :],
                                )
                            nc.scalar.dma_start(
                                rb[64 * h:64 * (h + 1), :, :],
                                drec[s][qs, :].rearrange(
                                    "g (o n) -> o g n", o=1
                                ).broadcast_to([64, NQ, NSLICE]),
                            )
                        nc.vector.tensor_mul(
                            xn_sb[s][:, NQ * q:NQ * (q + 1), :], aa[:], rb[:]
                        )
                    nc.vector.tensor_add(xn_sb[s][:, 0:NQ, :],
                                         xn_sb[s][:, 0:NQ, :],
                                         xn_sb[s][:, NQ:2 * NQ, :])

            # ---- phase 2: attention ------------------------------------
            # Passes (hp, nh); within a pass: both heads' scores into one
            # 2-bank PSUM pair, one merged Exp on ScalarE, 0/1 mask multiply
            # on VectorE, attn@[V|ones] accumulation into the [65,1024] vo
            # pair. Unnormalized output + denominator rows are evicted and
            # staged straight into the quad AllToAll.
            passes = [(hp, nh) for hp in range(2) for nh in range(NH)]
            DPP = HS // NSLICE       # dest chunks produced per pass

            def evict_stage(hp, nh, vop, on_scalar=False):
                # evict unnormalized y + denominator row 64 and stage into
                # the a2a chunks for this pass's token slice (both quads'
                # like-ranked peers get the same payload)
                for h in range(2):
                    ev = evpool.tile([65, HS], BF16, tag=f"ev{h}", name="ev")
                    if on_scalar:
                        nc.scalar.copy(ev[:], vop[:, 512 * h:512 * (h + 1)])
                    else:
                        with nc.allow_low_precision(reason="softmax y bf16"):
                            nc.vector.tensor_copy(
                                ev[:], vop[:, 512 * h:512 * (h + 1)]
                            )
                    for dd in range(DPP):
                        for dest in (nh * DPP + dd, nh * DPP + dd + NQ):
                            nc.sync.dma_start(
                                a2a_in[hp][130 * dest + 65 * h:
                                           130 * dest + 65 * (h + 1), :],
                                ev[:, NSLICE * dd:NSLICE * (dd + 1)],
                            )
                if nh == NH - 1:
                    nc.gpsimd.collective_compute(
                        "AllToAll",
                        mybir.AluOpType.bypass,
                        replica_groups=GROUPS,
                        ins=[a2a_in[hp][:]],
                        outs=[a2a_out[hp][:]],
                    )
                    if hp == 0:
                        # s=0 receiver preps: scalar-queue DMAs + schedule-
                        # pinned DVE ops overlap the later passes / a2a
                        preps(0, w=10.0)

            # software-pipelined across pass boundaries: tile t's attn@V is
            # emitted after tile t+1's scores (even across passes), so the
            # exp stream never drains at a pass boundary (the old per-pass
            # pipeline cost ~2.4us x 7 boundaries of ScalarE idle).
            prev = None          # (hp, nh, td, vop, pum)
            pend_ev = None       # completed pass awaiting evict, (args, gt)
            gt = 0               # global tile counter
            vop = None
            for hp, nh in passes:
                nsl = slice(HS * nh, HS * (nh + 1))
                vop = ps.tile([65, 1024], F32, tag="PC", name="vop")
                for t in range(MT):
                    # the last tile takes the third slot (PD) so the next
                    # pass's first score pairs (PA at t0, PB at t1) only
                    # wait on exps t14/t13 of this pass, which are done by
                    # the boundary -- the exp stream never drains
                    stag = "PD" if t == MT - 1 else ("PA", "PB")[t % 2]
                    spair = ps.tile([128, 1024], F32, tag=stag, name="s")
                    # score matmul pair at row groups (0,0)/(64,0) so the
                    # K=64 row-group concurrency engages
                    for h in range(2):
                        nc.tensor.matmul(
                            spair[:, 512 * h:512 * (h + 1)],
                            kt_sb[hp][64 * h:64 * (h + 1),
                                      128 * t:128 * (t + 1)],
                            qt_sb[hp][64 * h:64 * (h + 1), nsl],
                            start=True, stop=True,
                            tile_position=(64 * h, 0),
                        )
                    if prev is not None:
                        phn, pnh, td, pvop, pum_d = prev
                        for h in range(2):
                            nc.tensor.matmul(
                                pvop[:, 512 * h:512 * (h + 1)],
                                vt_sb[td][:, 2 * phn + h, :],
                                pum_d[:, h, :],
                                start=(td == 0), stop=(td == MT - 1),
                            )
                        if td == MT - 1:
                            pend_ev = ((phn, pnh, pvop), gt)
                    pu = pupool.tile([128, 1024], BF16, tag="pu", name="pu")
                    nc.scalar.activation(
                        pu[:], spair[:],
                        mybir.ActivationFunctionType.Exp,
                        scale=float(SCALE),
                    )
                    # multiplicative 0/1 mask (bf16 2x mode)
                    pum = pumpool.tile([128, 2, HS], BF16, tag="pum",
                                       name="pum")
                    if USE_MASK_BCAST:
                        mb_ = mask_sb[:, t, nsl].rearrange(
                            "p (o n) -> p o n", o=1
                        ).broadcast_to([128, 2, HS])
                        nc.vector.tensor_mul(
                            pum[:],
                            pu[:].rearrange("p (g n) -> p g n", g=2),
                            mb_,
                        )
                    else:
                        for h in range(2):
                            nc.vector.tensor_mul(
                                pum[:, h, :],
                                pu[:, 512 * h:512 * (h + 1)],
                                mask_sb[:, t, nsl],
                            )
                    prev = (hp, nh, t, vop, pum)
                    # evict the finished pass one tile later, so its copies
                    # don't sit between the boundary mask-muls on the DVE
                    # queue and stall the exp-feeding pipeline (~2.3us/pass)
                    if pend_ev is not None and pend_ev[1] < gt:
                        evict_stage(*pend_ev[0])
                        pend_ev = None
                    gt += 1
                    # produce the next Q chunk mid-pass (the PD slot is free)
                    if hp == 0 and t == 8 and nh + 1 < NH:
                        q_chunk(nh + 1)
            # drain the pipeline: last tile's attn@V + eviction (on the now-
            # idle scalar engine) + the second a2a launch
            if pend_ev is not None:
                evict_stage(*pend_ev[0])
                pend_ev = None
            phn, pnh, td, pvop, pum_d = prev
            for h in range(2):
                nc.tensor.matmul(
                    pvop[:, 512 * h:512 * (h + 1)],
                    vt_sb[td][:, 2 * phn + h, :],
                    pum_d[:, h, :],
                    start=(td == 0), stop=(td == MT - 1),
                )
            evict_stage(phn, pnh, pvop, on_scalar=True)

            # ---- phase 3: output projection ----------------------------
            # all NT row tiles accumulate concurrently: [128,1024] tiles in
            # PA/PB/PC plus the last tile split across PD1/PD2. The s=0
            # contributions run during the second a2a.
            pj = {nt: ps.tile([128, 1024], F32, tag=("PA", "PB", "PC")[nt % 3],
                              name=f"pj{nt}")
                  for nt in range(NT - 1)}
            ntl = NT - 1
            pjl_t = ps.tile([128, 1024], F32, tag="PD", name="pjl")
            pjl = [pjl_t[:, 512 * c:512 * (c + 1)] for c in range(2)]

            def proj_mms(s, start, stop):
                for nt in range(NT):
                    for j in range(NQ):
                        for ch in range(2):
                            dst = (pj[nt][:, 512 * ch:512 * (ch + 1)]
                                   if nt < ntl else pjl[ch])
                            nc.tensor.matmul(
                                dst,
                                xn_sb[s][:, j, 128 * nt:128 * (nt + 1)],
                                wp_sb[s][j][:, 512 * ch:512 * (ch + 1)],
                                start=(start and j == 0),
                                stop=(stop and j == NQ - 1),
                            )

            proj_mms(0, start=True, stop=False)
            preps(1, w=10.1)
            proj_mms(1, start=False, stop=True)
            for nt in range(NT):
                o_t = opool.tile([128, DIM], BF16, tag="ot", name="ot")
                with nc.allow_low_precision(reason="bf16 output"):
                    if nt < NT - 1:
                        nc.vector.tensor_add(o_t[:], pj[nt][:], bpr_sb[:])
                    else:
                        for ch in range(2):
                            csl = slice(512 * ch, 512 * (ch + 1))
                            nc.vector.tensor_add(o_t[:, csl], pjl[ch],
                                                 bpr_sb[:, csl])
                nc.sync.dma_start(out_e[128 * nt:128 * (nt + 1), :], o_t[:])

    if split_waits:
        _split_sync_waits(nc)
    return nc


def make_in_maps(q, k, v, mask, Wq, Wk, Wv, Wp, bp, N=N_FULL):
    """Shard + pre-transpose + bf16-cast the full inputs for the 8 cores."""
    bf = lambda a: np.ascontiguousarray(a).astype(BF16_NP)

    def pkt(a, inner):
        # [KT_D*128, inner] -> [128, KT_D*inner] so the device DMA of the
        # (p, kt, inner) view is contiguous
        return a.reshape(KT_D, 128, inner).transpose(1, 0, 2).reshape(128, -1)
    bp_rep = np.ascontiguousarray(
        np.broadcast_to(bp.astype(np.float32), (128, DIM))
    )
    # Wp rows permuted to the a2a_out block order: block (s, j) holds source
    # quad-rank j's head pair s = global heads {4j+2s, 4j+2s+1}, i.e. Wp rows
    # [128*(2j+s), 128*(2j+s)+128]. Same permutation for every core.
    wp_perm = np.empty((DIM, DIM), np.float32)
    for s in range(2):
        for j in range(NQ):
            wp_perm[128 * (s * NQ + j):128 * (s * NQ + j + 1)] = \
                Wp[128 * (2 * j + s):128 * (2 * j + s + 1)]
    in_maps = []
    for c in range(NCORES):
        b, r = divmod(c, 4)
        cs = slice(COLS * r, COLS * (r + 1))
        # per-partition quad/batch filter for the denominator reciprocals
        selc = np.zeros((16, 1), np.float32)
        for g in range(NCORES):
            if g // 4 == b:
                selc[2 * g:2 * g + 2] = 1.0
        in_maps.append({
            "selc": selc,
            # [p, nch, half, kt4, n] packing of q^T (see xqr_e)
            "xqr": bf(np.ascontiguousarray(q[b].T).reshape(
                2, KT_D // 2, 128, N // 512, 512
            ).transpose(2, 3, 0, 1, 4).reshape(128, -1)),
            "xkT": bf(k[b].T),
            "xvr": bf(pkt(np.ascontiguousarray(v[b].T), N)),
            "wq": bf(pkt(Wq[:, cs], COLS)),
            "wk": bf(pkt(Wk[:, cs], COLS)),
            "wv": bf(pkt(Wv[:, cs], COLS)),
            "wp_perm": bf(wp_perm),
            "maskT": bf(mask[b, 0].T.astype(np.float32)),
            "bp_rep": bp_rep,
        })
    return in_maps


def assemble_out(results, N=N_FULL):
    NSLICE = N // 4
    out = np.empty((B, N, DIM), np.float32)
    for c in range(NCORES):
        b, r = divmod(c, 4)
        out[b, NSLICE * r:NSLICE * (r + 1), :] = \
            results[c]["out"].astype(np.float32)
    return out


_NC_CACHE = {}


def _get_nc():
    if "nc" not in _NC_CACHE:
        _NC_CACHE["nc"] = build_nc()
    return _NC_CACHE["nc"]


def kernel(q, k, v, mask, Wq, Wk, Wv, Wp, bp):
    from concourse.bass_utils import run_bass_kernel_spmd

    q, k, v = (np.asarray(a, np.float32) for a in (q, k, v))
    mask = np.asarray(mask)
    Wq, Wk, Wv, Wp, bp = (
        np.asarray(a, np.float32) for a in (Wq, Wk, Wv, Wp, bp)
    )
    nc = _get_nc()
    in_maps = make_in_maps(q, k, v, mask, Wq, Wk, Wv, Wp, bp)
    res = run_bass_kernel_spmd(nc, in_maps, core_ids=list(range(NCORES)))
    return assemble_out(res.results)


# revision 66
# speedup vs baseline: 1.0291x; 1.0291x over previous
"""Multi-head attention (B=2, N=2048, DIM=1024, H=16) on 8 Trainium2 NeuronCores.

Sharding: tensor-parallel by head within two quads (cores 0-3 -> batch 0,
cores 4-7 -> batch 1; quad rank r owns heads 4r..4r+3). Each core computes
Q/K/V projections for its 4 heads and masked-softmax attention; per-quad
(4-core) AllToAlls re-shard the attention output from head-split to
sequence-split; each core runs the output projection (+bias) for a disjoint
512-token slice. The host only shards inputs (transpose + bf16 cast) and
concatenates the 8 output shards.

v3 structure (vs the 461us v2):
  - quad-local AllToAlls (replica groups [[0-3],[4-7]]): half the collective
    payload, and the output projection contracts only the 8 real source
    blocks (v2 contracted 16, half against zero-padded weights).
  - the a2a payload is the UNNORMALIZED attention sum plus the softmax
    denominator row (the extra ones-column of the attn@V matmul);
    normalization happens on the receiver with reciprocal_approx_fast +
    a tiny selector-matmul partition-broadcast. This removes the whole
    normalization chain (reciprocals were 56us of DVE) from the attention
    passes and lets each a2a launch one pass earlier.
  - merged exp: the two heads of a pass share one [128,1024] PSUM score
    pair (2 banks), so ExpE runs half as many, double-size activations.
  - receiver-side prep for the first a2a is interleaved into the later
    attention passes; the tail is only the second a2a + projection matmuls.

Numerics: matmuls in bf16 with fp32 PSUM accumulation; softmax computed as
exp(SCALE*S)*mask / sum(exp(SCALE*S)*mask) without max-subtraction (scores
are ~N(0,1) after SCALE; exp never overflows). Denominators come from an
extra ones-column appended to V in the attn@V matmul (column 64 of each
head's [128,65] V tile); they travel through the a2a in bf16 (0.4% rel
error, well under the 2e-2 gate).
"""

import numpy as np
import ml_dtypes

import concourse.bass as bass
import concourse.mybir as mybir
import concourse.tile as tile

F32 = mybir.dt.float32
BF16 = mybir.dt.bfloat16
BF16_NP = ml_dtypes.bfloat16

B, DIM, H = 2, 1024, 16
N_FULL = 2048
HD = DIM // H          # 64
SCALE = HD ** -0.5     # 0.125
NCORES = 8
H_LOC = H // 4         # 4 heads per core
COLS = H_LOC * HD      # 256 local channels
KT_D = DIM // 128      # 8 contraction tiles over DIM
GROUPS = [list(range(NCORES))]   # NRT mesh needs >4 cores per group
NQ = 4                 # quad size
USE_MASK_BCAST = True  # single [128,2,HS] mask multiply via stride-0 view


# ---------------------------------------------------------------------------
# Workaround: this walrus build rejects >2 sync waits on one instruction
# ("Too many sync wait commands" in setupSyncWait). The TileContext final
# drain aggregates one wait per logical processor; split it into a chain of
# single-wait drains.
# ---------------------------------------------------------------------------
def _patch_tile_drain():
    from bass_rust import ScopedClock

    if getattr(tile.TileContext, "_drain_patched", False):
        return

    def _drain_and_barrier(self, tick_clock, wait_clock):
        nc = self.nc
        drain_inst = nc.sync.drain()
        wait_clock.add_sem_waits(
            drain_inst.ins, ScopedClock({None: tick_clock.global_clock})
        )
        si = drain_inst.ins.sync_info
        if si is not None and len(si.on_wait) > 1:
            waits = list(si.on_wait)
            drain_inst.ins.sync_info = mybir.SyncInfo(
                on_wait=waits[:1], on_update=list(si.on_update)
            )
            for w in waits[1:]:
                d = nc.sync.drain()
                dsi = d.ins.sync_info
                upd = list(dsi.on_update) if dsi is not None else []
                d.ins.sync_info = mybir.SyncInfo(on_wait=[w], on_update=upd)

        nc.all_engine_barrier()
        assert self.sems is not None
        popped = nc._tile_sem_poison_stack.pop()
        assert popped is self._sem_poison
        nc.clear_and_free_semaphores(list(self.sems.allocated().values()))
        nc.all_engine_barrier()

    tile.TileContext._drain_and_barrier = _drain_and_barrier
    tile.TileContext._drain_patched = True


def _split_sync_waits(nc, maxw=1):
    """Walrus in this build rejects instructions carrying more than a couple
    of semaphore waits. Move excess waits onto injected same-engine NoOps
    immediately before the instruction (identical semantics: the engine
    blocks at the nop instead of at the instruction itself)."""
    n_split = 0
    for f in nc.m.functions:
        for bb in f.blocks:
            new_insts = []
            for ins in bb.instructions:
                si = ins.sync_info
                if si is not None and len(si.on_wait) > maxw:
                    waits = list(si.on_wait)
                    for i, w in enumerate(waits[maxw:]):
                        nop = mybir.InstNoOp(
                            name=f"{ins.name}-w{i}", ins=[], outs=[]
                        )
                        nop.engine = ins.engine
                        nop.sync_info = mybir.SyncInfo(
                            on_wait=[w], on_update=[]
                        )
                        new_insts.append(nop)
                    ins.sync_info = mybir.SyncInfo(
                        on_wait=waits[:maxw], on_update=list(si.on_update)
                    )
                    n_split += 1
                new_insts.append(ins)
            bb.instructions = new_insts
    return n_split


def build_nc(N=N_FULL, split_waits=True):
    """Build the per-core Bass program (same SPMD program for all 8 cores).

    N is parameterizable (multiple of 512) so a scaled-down variant can be
    validated in the simulator; the graded configuration is N=2048.
    """
    _patch_tile_drain()
    assert N % 512 == 0
    NSLICE = N // 4            # output rows per core
    MT = N // 128              # m-tiles over keys
    HS = 512                   # attention n-chunk size
    NH = N // HS               # number of n-chunks per head pair
    NT = NSLICE // 128         # output row tiles
    NCH = N // 512             # 512-col chunks of N

    nc = bass.Bass(trn_type="TRN2", num_devices=NCORES)

    # x_q host-packed as [p, nch, half, kt4, n] so each Q-chunk DMA is
    # contiguous per partition (column-sliced reads of x^T would move in
    # 1KB granules and run ~4x slower)
    xqr_e = nc.declare_dram_parameter("xqr", [128, KT_D * N], BF16,
                                      isOutput=False)
    xkT_e = nc.declare_dram_parameter("xkT", [DIM, N], BF16, isOutput=False)
    # weights host-rearranged to [p, kt, c] so their DMA is contiguous
    wq_e = nc.declare_dram_parameter("wq", [128, KT_D * COLS], BF16,
                                     isOutput=False)
    wk_e = nc.declare_dram_parameter("wk", [128, KT_D * COLS], BF16,
                                     isOutput=False)
    wv_e = nc.declare_dram_parameter("wv", [128, KT_D * COLS], BF16,
                                     isOutput=False)
    # x_v host-rearranged to [p, kt, n] (contiguous DMA)
    xvr_e = nc.declare_dram_parameter("xvr", [128, KT_D * N], BF16,
                                      isOutput=False)
    # Wp rows permuted host-side into [(s,j) block, 128, DIM]: block (s,j)
    # holds the rows for source quad-rank j's head pair s. Source cores j
    # and j+4 share the block (same heads, different batch) -- the per-core
    # `sel` selector zeroes the cross-quad (wrong-batch) contributions.
    wpp_e = nc.declare_dram_parameter("wp_perm", [DIM, DIM], BF16,
                                      isOutput=False)
    # per-partition quad/batch filter for the denominator reciprocals: row
    # 2g+h is 1.0 iff source core g is in this core's quad. Per-core data;
    # the program stays SPMD.
    selc_e = nc.declare_dram_parameter("selc", [16, 1], F32, isOutput=False)
    maskT_e = nc.declare_dram_parameter("maskT", [N, N], BF16, isOutput=False)
    bpr_e = nc.declare_dram_parameter("bp_rep", [128, DIM], F32, isOutput=False)
    # bf16 output (0.2% quantization, well under the 2e-2 gate) halves the
    # output-DMA bytes on the tail critical path; the host upcasts
    out_e = nc.declare_dram_parameter("out", [NSLICE, DIM], BF16,
                                      isOutput=True)

    # per-head-pair AllToAll buffers: 8 peers x (2 heads x 65 rows); the
    # chunk for peers d and d+4 carries the same payload (only the same-quad
    # copy is used downstream).
    a2a_in = [nc.dram_tensor(f"a2a_in{s}", [NCORES * 130, NSLICE], BF16)
              for s in range(2)]
    a2a_out = [nc.dram_tensor(f"a2a_out{s}", [NCORES * 130, NSLICE], BF16)
               for s in range(2)]
    # DRAM staging for the zeroed denominator reciprocals: a stride-0 DMA
    # from here broadcasts each row over 64 SBUF partitions
    drec = [nc.dram_tensor(f"drec{s}", [16, NSLICE], BF16) for s in range(2)]

    with tile.TileContext(nc) as tc:
        with (
            tc.tile_pool(name="cpool", bufs=1) as cpool,
            tc.tile_pool(name="xstream", bufs=3) as xpool,
            tc.tile_pool(name="pupool", bufs=3) as pupool,
            tc.tile_pool(name="pumpool", bufs=4) as pumpool,
            tc.tile_pool(name="evpool", bufs=2) as evpool,
            tc.tile_pool(name="p3pool", bufs=2) as p3pool,
            tc.tile_pool(name="opool", bufs=2) as opool,
            tc.tile_pool(name="ps", bufs=1, space="PSUM") as ps,
        ):
            # PSUM: 16KB/partition as five slots -- PA/PB/PC are 2-bank
            # [128,1024] slots, PD1/PD2 single banks. Phase 1 uses all five
            # as projection accumulators; phase 2 uses PA/PB as the 2-deep
            # score-pair rotation (both heads side by side, so one Exp covers
            # them), PC as the attn@V accumulator pair, PD1/PD2 for the
            # interleaved receiver-side normalization broadcasts; phase 3
            # uses PA/PB/PC as output-projection accumulators and PD1/PD2
            # for the last row tile.
            # ---- long-lived SBUF tensors -------------------------------
            qt_sb = [cpool.tile([128, N], BF16, tag=f"qt{i}", name=f"qt{i}")
                     for i in range(2)]
            kt_sb = [cpool.tile([128, N], BF16, tag=f"kt{i}", name=f"kt{i}")
                     for i in range(2)]
            # V per m-tile: [m, head, 65]; cols 0..63 = V_head, col 64 = ones
            vt_sb = [cpool.tile([128, H_LOC, 65], BF16, tag=f"vt{t}",
                                name=f"vt{t}")
                     for t in range(MT)]
            mask_sb = cpool.tile([128, MT, N], BF16, tag="mask", name="mask")
            bpr_sb = cpool.tile([128, DIM], F32, tag="bpr", name="bpr")
            wq_sb = cpool.tile([128, KT_D, COLS], BF16, tag="wq", name="wq")
            wk_sb = cpool.tile([128, KT_D, COLS], BF16, tag="wk", name="wk")
            wv_sb = cpool.tile([128, KT_D, COLS], BF16, tag="wv", name="wv")
            xv_sb = cpool.tile([128, KT_D, N], BF16, tag="xv", name="xv")
            selc_sb = cpool.tile([16, 1], F32, tag="selc", name="selc")
            # phase-3 per-(s,src) normalized x, batched denominator recips,
            # and the 8 distinct Wp blocks
            xn_sb = [cpool.tile([128, NCORES, NSLICE], BF16, tag=f"xn{s}",
                                name=f"xn{s}") for s in range(2)]
            recb_sb = [cpool.tile([16, NSLICE], BF16, tag=f"recb{s}",
                                  name=f"recb{s}") for s in range(2)]
            # the 8 Wp blocks live in xv_sb's space (xv is dead after the
            # V projection; subtile WAR deps order the overwrite)
            if KT_D * N >= 8 * DIM:
                xv_flat = xv_sb[:].rearrange("p k n -> p (k n)")
                wp_sb = [[xv_flat[:, (s * NQ + j) * DIM:
                                  (s * NQ + j + 1) * DIM]
                          for j in range(NQ)] for s in range(2)]
            else:   # scaled-down sim configs have SBUF to spare
                wp_sb = [[cpool.tile([128, DIM], BF16, tag=f"wp{s}{j}",
                                     name=f"wp{s}{j}")
                          for j in range(NQ)] for s in range(2)]

            # weights + constants (wq prefetches during the K round; wv/bpr
            # are emitted later so they don't delay the x streams)
            nc.sync.dma_start(wk_sb[:], wk_e[:].rearrange("p (kt c) -> p kt c", kt=KT_D))
            nc.sync.dma_start(wq_sb[:], wq_e[:].rearrange("p (kt c) -> p kt c", kt=KT_D))
            nc.sync.dma_start(selc_sb[:], selc_e[:])
            for t in range(MT):
                nc.gpsimd.memset(vt_sb[t][:, :, 64:65], 1.0)

            # ---- phase 1: projections ----------------------------------
            # PSUM accumulator slots: (tag, shape, col-offset) covering the
            # 8 [128,512] K/Q accumulators (cb x nch).
            def qk_psum_slots():
                tiles = {}
                def slot(i):
                    tag, off = [("PA", 0), ("PA", 512), ("PB", 0), ("PB", 512),
                                ("PC", 0), ("PC", 512), ("PD", 0), ("PD", 512)][i]
                    if tag not in tiles:
                        tiles[tag] = ps.tile([128, 1024], F32, tag=tag,
                                             name="p1qk")
                    return tiles[tag][:, off:off + 512]
                return slot

            # K^T: [COLS, N] as two 128-row blocks; kt-outer with one live
            # [128, 512] psum accumulator per (block, n-chunk).
            slot = qk_psum_slots()
            psums = [slot(cb * NCH + nch)
                     for cb in range(2) for nch in range(NCH)]
            for kt in range(KT_D):
                xt_t = xpool.tile([128, N], BF16, tag="xs", name="xs")
                nc.sync.dma_start(xt_t[:], xkT_e[128 * kt:128 * (kt + 1), :])
                for cb in range(2):
                    for nch in range(NCH):
                        nc.tensor.matmul(
                            psums[cb * NCH + nch],
                            wk_sb[:, kt, 128 * cb:128 * (cb + 1)],
                            xt_t[:, 512 * nch:512 * (nch + 1)],
                            start=(kt == 0), stop=(kt == KT_D - 1),
                        )
            for cb in range(2):
                for nch in range(NCH):
                    nc.scalar.copy(
                        kt_sb[cb][:, 512 * nch:512 * (nch + 1)],
                        psums[cb * NCH + nch],
                    )

            # Q^T: n-chunk-outer (x re-read as column slices, same total
            # bytes) so chunk 0 is ready right after K/V and the attention
            # passes can start; chunks k>=1 are emitted inside pass (0,k-1)
            # where PD1/PD2 and tensor slack are free.
            def q_chunk(nch, qtag="PD"):
                qps = ps.tile([128, 1024], F32, tag=qtag, name="qps")
                for half in range(2):
                    xq_t = xpool.tile([128, KT_D // 2, 512], BF16, tag="xs",
                                      name="xq")
                    blk = 2 * nch + half
                    nc.sync.dma_start(
                        xq_t[:],
                        xqr_e[:, 2048 * blk:2048 * (blk + 1)].rearrange(
                            "p (kt n) -> p kt n", kt=KT_D // 2
                        ),
                    )
                    for cb in range(2):
                        for k4 in range(KT_D // 2):
                            kt = half * (KT_D // 2) + k4
                            nc.tensor.matmul(
                                qps[:, 512 * cb:512 * (cb + 1)],
                                wq_sb[:, kt, 128 * cb:128 * (cb + 1)],
                                xq_t[:, k4, :],
                                start=(kt == 0), stop=(kt == KT_D - 1),
                            )
                for cb in range(2):
                    with nc.allow_low_precision(reason="qt bf16"):
                        nc.vector.tensor_copy(
                            qt_sb[cb][:, 512 * nch:512 * (nch + 1)],
                            qps[:, 512 * cb:512 * (cb + 1)],
                        )

            q_chunk(0)

            # V in natural layout: out[m-tile, 4*HD] = xvT_kt^T @ wv_kt
            nc.sync.dma_start(
                wv_sb[:], wv_e[:].rearrange("p (kt c) -> p kt c", kt=KT_D)
            )
            # per-kt slices so the V matmuls start as soon as the first
            # contraction slice lands instead of after the whole 4MB
            for kt in range(KT_D):
                nc.sync.dma_start(
                    xv_sb[:, kt, :], xvr_e[:, N * kt:N * (kt + 1)]
                )
            for t in range(MT):
                nc.sync.dma_start(
                    mask_sb[:, t, :], maskT_e[128 * t:128 * (t + 1), :]
                )
            nc.sync.dma_start(bpr_sb[:], bpr_e[:])
            for t in range(MT):
                vps = ps.tile([128, COLS], F32, tag=("PA", "PB")[t % 2],
                              name="p1v")
                for kt in range(KT_D):
                    nc.tensor.matmul(
                        vps[:],
                        xv_sb[:, kt, 128 * t:128 * (t + 1)],
                        wv_sb[:, kt, :],
                        start=(kt == 0), stop=(kt == KT_D - 1),
                    )
                nc.scalar.copy(
                    vt_sb[t][:, :, 0:HD],
                    vps[:].rearrange("p (h d) -> p h d", h=H_LOC),
                )

            # phase-3 Wp blocks: DMA them up front (deadline is the tail)
            wpp_v = wpp_e[:].rearrange("(b p) c -> b p c", p=128)
            for s in range(2):
                for j in range(NQ):
                    nc.sync.dma_start(wp_sb[s][j][:], wpp_v[s * NQ + j])

            # ---- receiver-side prep for the a2a payload ----------------
            # prep_dn_a: one strided DMA gathers all 16 denominator rows
            # (every 65th row of a2a_out) + one batched [16,NSLICE]
            # reciprocal (replaces the baseline's 32 single-partition
            # reciprocals, 55us of DVE).
            # prep_dn_b: downcast with the per-partition quad filter (cross-
            # quad rows -> 0) and stage to DRAM for the broadcast DMAs.
            # prep_block: one DMA for the block's 128 x rows, one stride-0
            # broadcast DMA for its two recip rows, one 2x-mode multiply.
            # prep_sum folds the (wrong-batch, zeroed) twin into the
            # quad-local tile so the projection contracts 8 blocks.
            # prep DMAs issue from the (otherwise idle) GpSimd queue: if the
            # collective runs long, only GpSimd stalls on it -- the Sync
            # queue keeps issuing the passes' staging DMAs. The DVE ops are
            # pinned to the end of the schedule with tile_wait_until: the
            # Tile scheduler's cost model assumes collectives are fast and
            # would otherwise hoist them into the middle of the attention
            # passes, where their data dependency on the in-flight a2a
            # stalls the in-order DVE queue (and with it exp/mask/attnV).
            def W(ms):
                return tc.tile_wait_until(ms)

            def preps(s, w):
                # batched receiver-side prep: 1 strided DMA for the 16
                # denominator rows, one [16,NSLICE] reciprocal (vs the
                # baseline's 32 single-partition ones), zeroed bf16 downcast
                # (per-partition quad filter), DRAM roundtrip, then two
                # 4-block rounds of {x halves + stride-0 recip broadcast}
                # DMAs, each closed by one [128,4,NSLICE] multiply.
                srcv = a2a_out[s][:].rearrange("(q r) n -> q r n", r=65)
                srcp = srcv.rearrange("q r n -> r q n")
                dn = p3pool.tile([16, NSLICE], BF16, tag="dn", name="dn",
                                 bufs=1)
                rec = p3pool.tile([16, NSLICE], F32, tag="rec", name="rec",
                                  bufs=1)
                with W(w):
                    nc.scalar.dma_start(dn[:], srcv[:, 64, :])
                    nc.vector.reciprocal(rec[:], dn[:])
                    with nc.allow_low_precision(reason="softmax recip bf16"):
                        nc.vector.tensor_scalar_mul(recb_sb[s][:], rec[:],
                                                    selc_sb[:])
                    aa0 = p3pool.tile([128, NQ, NSLICE], BF16, tag="aa",
                                      name="aa", bufs=1)
                    for h in range(2):   # x stream starts during the recip
                        nc.scalar.dma_start(
                            aa0[64 * h:64 * (h + 1), :, :],
                            srcp[0:64, slice(h, 8, 2), :],
                        )
                    nc.scalar.dma_start(drec[s][:], recb_sb[s][:])
                    for q in range(2):
                        aa = aa0 if q == 0 else p3pool.tile(
                            [128, NQ, NSLICE], BF16, tag="aa", name="aa",
                            bufs=1)
                        rb = p3pool.tile([128, NQ, NSLICE], BF16, tag="rb",
                                         name="rb", bufs=1)
                        for h in range(2):
                            qs = slice(8 * q + h, 8 * q + 8, 2)
                            if q == 1:
                                nc.scalar.dma_start(
                                    aa[64 * h:64 * (h + 1), :, :],
                                    srcp[0:64, qs, :],
                                )
                            nc.scalar.dma_start(
                                rb[64 * h:64 * (h + 1), :, :],
                                drec[s][qs, :].rearrange(
                                    "g (o n) -> o g n", o=1
                                ).broadcast_to([64, NQ, NSLICE]),
                            )
                        nc.vector.tensor_mul(
                            xn_sb[s][:, NQ * q:NQ * (q + 1), :], aa[:], rb[:]
                        )
                    nc.vector.tensor_add(xn_sb[s][:, 0:NQ, :],
                                         xn_sb[s][:, 0:NQ, :],
                                         xn_sb[s][:, NQ:2 * NQ, :])

            # ---- phase 2: attention ------------------------------------
            # Passes (hp, nh); within a pass: both heads' scores into one
            # 2-bank PSUM pair, one merged Exp on ScalarE, 0/1 mask multiply
            # on VectorE, attn@[V|ones] accumulation into the [65,1024] vo
            # pair. Unnormalized output + denominator rows are evicted and
            # staged straight into the quad AllToAll.
            passes = [(hp, nh) for hp in range(2) for nh in range(NH)]
            DPP = HS // NSLICE       # dest chunks produced per pass

            def evict_stage(hp, nh, vop, on_scalar=False):
                # evict unnormalized y + denominator row 64 and stage into
                # the a2a chunks for this pass's token slice (both quads'
                # like-ranked peers get the same payload)
                for h in range(2):
                    ev = evpool.tile([65, HS], BF16, tag=f"ev{h}", name="ev")
                    if on_scalar:
                        nc.scalar.copy(ev[:], vop[:, 512 * h:512 * (h + 1)])
                    else:
                        with nc.allow_low_precision(reason="softmax y bf16"):
                            nc.vector.tensor_copy(
                                ev[:], vop[:, 512 * h:512 * (h + 1)]
                            )
                    for dd in range(DPP):
                        for dest in (nh * DPP + dd, nh * DPP + dd + NQ):
                            nc.sync.dma_start(
                                a2a_in[hp][130 * dest + 65 * h:
                                           130 * dest + 65 * (h + 1), :],
                                ev[:, NSLICE * dd:NSLICE * (dd + 1)],
                            )
                if nh == NH - 1:
                    nc.gpsimd.collective_compute(
                        "AllToAll",
                        mybir.AluOpType.bypass,
                        replica_groups=GROUPS,
                        ins=[a2a_in[hp][:]],
                        outs=[a2a_out[hp][:]],
                    )
                    if hp == 0:
                        # s=0 receiver preps: scalar-queue DMAs + schedule-
                        # pinned DVE ops overlap the later passes / a2a
                        preps(0, w=10.0)

            # software-pipelined across pass boundaries: tile t's attn@V is
            # emitted after tile t+1's scores (even across passes), so the
            # exp stream never drains at a pass boundary (the old per-pass
            # pipeline cost ~2.4us x 7 boundaries of ScalarE idle).
            prev = None          # (hp, nh, td, vop, pum)
            pend_ev = None       # completed pass awaiting evict, (args, gt)
            gt = 0               # global tile counter
            vop = None
            for pidx, (hp, nh) in enumerate(passes):
                nsl = slice(HS * nh, HS * (nh + 1))
                # alternate the attn@V accumulator bank pair so pass i+1's
                # first attn@V never waits on pass i's eviction copies
                vop = ps.tile([65, 1024], F32, tag=("PC", "PD")[pidx % 2],
                              name="vop")
                for t in range(MT):
                    spair = ps.tile([128, 1024], F32,
                                    tag=("PA", "PB")[t % 2], name="s")
                    # score matmul pair at row groups (0,0)/(64,0) so the
                    # K=64 row-group concurrency engages
                    for h in range(2):
                        nc.tensor.matmul(
                            spair[:, 512 * h:512 * (h + 1)],
                            kt_sb[hp][64 * h:64 * (h + 1),
                                      128 * t:128 * (t + 1)],
                            qt_sb[hp][64 * h:64 * (h + 1), nsl],
                            start=True, stop=True,
                            tile_position=(64 * h, 0),
                        )
                    if prev is not None:
                        phn, pnh, td, pvop, pum_d = prev
                        for h in range(2):
                            nc.tensor.matmul(
                                pvop[:, 512 * h:512 * (h + 1)],
                                vt_sb[td][:, 2 * phn + h, :],
                                pum_d[:, h, :],
                                start=(td == 0), stop=(td == MT - 1),
                            )
                        if td == MT - 1:
                            pend_ev = ((phn, pnh, pvop), gt)
                    pu = pupool.tile([128, 1024], BF16, tag="pu", name="pu")
                    nc.scalar.activation(
                        pu[:], spair[:],
                        mybir.ActivationFunctionType.Exp,
                        scale=float(SCALE),
                    )
                    # multiplicative 0/1 mask (bf16 2x mode)
                    pum = pumpool.tile([128, 2, HS], BF16, tag="pum",
                                       name="pum")
                    if USE_MASK_BCAST:
                        mb_ = mask_sb[:, t, nsl].rearrange(
                            "p (o n) -> p o n", o=1
                        ).broadcast_to([128, 2, HS])
                        nc.vector.tensor_mul(
                            pum[:],
                            pu[:].rearrange("p (g n) -> p g n", g=2),
                            mb_,
                        )
                    else:
                        for h in range(2):
                            nc.vector.tensor_mul(
                                pum[:, h, :],
                                pu[:, 512 * h:512 * (h + 1)],
                                mask_sb[:, t, nsl],
                            )
                    prev = (hp, nh, t, vop, pum)
                    # evict the finished pass one tile later, so its copies
                    # don't sit between the boundary mask-muls on the DVE
                    # queue and stall the exp-feeding pipeline (~2.3us/pass)
                    if pend_ev is not None and pend_ev[1] < gt:
                        evict_stage(*pend_ev[0])
                        pend_ev = None
                    gt += 1
                    # produce the next Q chunk mid-pass, in whichever of
                    # PC/PD this pass's vop is not occupying
                    if hp == 0 and t == 8 and nh + 1 < NH:
                        q_chunk(nh + 1, qtag=("PD", "PC")[pidx % 2])
            # drain the pipeline: last tile's attn@V + eviction (on the now-
            # idle scalar engine) + the second a2a launch
            if pend_ev is not None:
                evict_stage(*pend_ev[0])
                pend_ev = None
            phn, pnh, td, pvop, pum_d = prev
            for h in range(2):
                nc.tensor.matmul(
                    pvop[:, 512 * h:512 * (h + 1)],
                    vt_sb[td][:, 2 * phn + h, :],
                    pum_d[:, h, :],
                    start=(td == 0), stop=(td == MT - 1),
                )
            evict_stage(phn, pnh, pvop, on_scalar=True)

            # ---- phase 3: output projection ----------------------------
            # all NT row tiles accumulate concurrently: [128,1024] tiles in
            # PA/PB/PC plus the last tile split across PD1/PD2. The s=0
            # contributions run during the second a2a.
            pj = {nt: ps.tile([128, 1024], F32, tag=("PA", "PB", "PC")[nt % 3],
                              name=f"pj{nt}")
                  for nt in range(NT - 1)}
            ntl = NT - 1
            pjl_t = ps.tile([128, 1024], F32, tag="PD", name="pjl")
            pjl = [pjl_t[:, 512 * c:512 * (c + 1)] for c in range(2)]

            def proj_mms(s, start, stop):
                for nt in range(NT):
                    for j in range(NQ):
                        for ch in range(2):
                            dst = (pj[nt][:, 512 * ch:512 * (ch + 1)]
                                   if nt < ntl else pjl[ch])
                            nc.tensor.matmul(
                                dst,
                                xn_sb[s][:, j, 128 * nt:128 * (nt + 1)],
                                wp_sb[s][j][:, 512 * ch:512 * (ch + 1)],
                                start=(start and j == 0),
                                stop=(stop and j == NQ - 1),
                            )

            proj_mms(0, start=True, stop=False)
            preps(1, w=10.1)
            proj_mms(1, start=False, stop=True)
            for nt in range(NT):
                o_t = opool.tile([128, DIM], BF16, tag="ot", name="ot")
                with nc.allow_low_precision(reason="bf16 output"):
                    if nt < NT - 1:
                        nc.vector.tensor_add(o_t[:], pj[nt][:], bpr_sb[:])
                    else:
                        for ch in range(2):
                            csl = slice(512 * ch, 512 * (ch + 1))
                            nc.vector.tensor_add(o_t[:, csl], pjl[ch],
                                                 bpr_sb[:, csl])
                nc.sync.dma_start(out_e[128 * nt:128 * (nt + 1), :], o_t[:])

    if split_waits:
        _split_sync_waits(nc)
    return nc


def make_in_maps(q, k, v, mask, Wq, Wk, Wv, Wp, bp, N=N_FULL):
    """Shard + pre-transpose + bf16-cast the full inputs for the 8 cores."""
    bf = lambda a: np.ascontiguousarray(a).astype(BF16_NP)

    def pkt(a, inner):
        # [KT_D*128, inner] -> [128, KT_D*inner] so the device DMA of the
        # (p, kt, inner) view is contiguous
        return a.reshape(KT_D, 128, inner).transpose(1, 0, 2).reshape(128, -1)
    bp_rep = np.ascontiguousarray(
        np.broadcast_to(bp.astype(np.float32), (128, DIM))
    )
    # Wp rows permuted to the a2a_out block order: block (s, j) holds source
    # quad-rank j's head pair s = global heads {4j+2s, 4j+2s+1}, i.e. Wp rows
    # [128*(2j+s), 128*(2j+s)+128]. Same permutation for every core.
    wp_perm = np.empty((DIM, DIM), np.float32)
    for s in range(2):
        for j in range(NQ):
            wp_perm[128 * (s * NQ + j):128 * (s * NQ + j + 1)] = \
                Wp[128 * (2 * j + s):128 * (2 * j + s + 1)]
    in_maps = []
    for c in range(NCORES):
        b, r = divmod(c, 4)
        cs = slice(COLS * r, COLS * (r + 1))
        # per-partition quad/batch filter for the denominator reciprocals
        selc = np.zeros((16, 1), np.float32)
        for g in range(NCORES):
            if g // 4 == b:
                selc[2 * g:2 * g + 2] = 1.0
        in_maps.append({
            "selc": selc,
            # [p, nch, half, kt4, n] packing of q^T (see xqr_e)
            "xqr": bf(np.ascontiguousarray(q[b].T).reshape(
                2, KT_D // 2, 128, N // 512, 512
            ).transpose(2, 3, 0, 1, 4).reshape(128, -1)),
            "xkT": bf(k[b].T),
            "xvr": bf(pkt(np.ascontiguousarray(v[b].T), N)),
            "wq": bf(pkt(Wq[:, cs], COLS)),
            "wk": bf(pkt(Wk[:, cs], COLS)),
            "wv": bf(pkt(Wv[:, cs], COLS)),
            "wp_perm": bf(wp_perm),
            "maskT": bf(mask[b, 0].T.astype(np.float32)),
            "bp_rep": bp_rep,
        })
    return in_maps


def assemble_out(results, N=N_FULL):
    NSLICE = N // 4
    out = np.empty((B, N, DIM), np.float32)
    for c in range(NCORES):
        b, r = divmod(c, 4)
        out[b, NSLICE * r:NSLICE * (r + 1), :] = \
            results[c]["out"].astype(np.float32)
    return out


_NC_CACHE = {}


def _get_nc():
    if "nc" not in _NC_CACHE:
        _NC_CACHE["nc"] = build_nc()
    return _NC_CACHE["nc"]


def kernel(q, k, v, mask, Wq, Wk, Wv, Wp, bp):
    from concourse.bass_utils import run_bass_kernel_spmd

    q, k, v = (np.asarray(a, np.float32) for a in (q, k, v))
    mask = np.asarray(mask)
    Wq, Wk, Wv, Wp, bp = (
        np.asarray(a, np.float32) for a in (Wq, Wk, Wv, Wp, bp)
    )
    nc = _get_nc()
    in_maps = make_in_maps(q, k, v, mask, Wq, Wk, Wv, Wp, bp)
    res = run_bass_kernel_spmd(nc, in_maps, core_ids=list(range(NCORES)))
    return assemble_out(res.results)
